# revision 1
# baseline (speedup 1.0000x reference)
"""Trainium2 Bass kernel for KAN Fourier linear layer.

y[b, j] = sum_{i,k} cos((k+1) x[b,i]) W0[j,i,k] + sin((k+1) x[b,i]) W1[j,i,k] + bias[j]

Strategy (8 cores, data-parallel over batch):
  - Each core handles B=1024 batch rows; fouriercoeffs replicated.
  - Host pre-transposes x to x^T (i on partitions) and reorders W to
    [contraction, j] with contraction order (trig, k, i) so each 128-row
    contraction chunk is (trig, k, i_half) = one ACT tile's worth.
  - Device per chunk: DVE range reduction g = (k/(2pi) * x + 0.5) mod 1,
    ACT Sin(2pi*g - pi) = sin(k x) (and +0.25 shift for cos), cast to bf16,
    then PE accumulates y^T[j, b] in PSUM over all 256 chunks.
  - W quantized to bf16 on host (halves DMA; error ~2e-3 rms on y).
"""

import numpy as np
import ml_dtypes

import concourse.bacc as bacc
import concourse.mybir as mybir
import concourse.tile as tile
from concourse import bass_utils

N_CORES = 8
B_FULL = 8192
B = B_FULL // N_CORES  # 1024 batch rows per core
I = 256
K = 64
J = 256
P = 128

_cache = {}


def _build():
    if "nc" in _cache:
        return _cache["nc"]

    f32 = mybir.dt.float32
    bf16 = mybir.dt.bfloat16
    nc = bacc.Bacc("TRN2", target_bir_lowering=False, debug=False, num_devices=N_CORES)

    xT_dram = nc.dram_tensor("xT", (I, B), f32, kind="ExternalInput")
    w_dram = nc.dram_tensor("w", (2 * I * K, J), bf16, kind="ExternalInput")
    bias_dram = nc.dram_tensor("bias", (J, 1), f32, kind="ExternalInput")
    yT_dram = nc.dram_tensor("yT", (J, B), f32, kind="ExternalOutput")

    TWO_PI = float(2.0 * np.pi)
    PI = float(np.pi)
    MAGIC = float(1.5 * 2.0**23)
    Alu = mybir.AluOpType
    Act = mybir.ActivationFunctionType

    with tile.TileContext(nc) as tc:
        with (
            tc.tile_pool(name="const", bufs=1) as const_pool,
            tc.tile_pool(name="wload", bufs=8) as w_pool,
            tc.tile_pool(name="red", bufs=3) as red_pool,
            tc.tile_pool(name="trig", bufs=4) as trig_pool,
            tc.tile_pool(name="psum", bufs=1, space="PSUM") as psum_pool,
            tc.tile_pool(name="out", bufs=2) as out_pool,
        ):
            # Constants: x^T halves (i on partitions), bias per-partition columns
            xT_sb = []
            bias_sb = []
            for h in range(2):
                xt = const_pool.tile([P, B], f32, tag=f"xT{h}")
                nc.sync.dma_start(xt[:], xT_dram[h * P : (h + 1) * P, :])
                xT_sb.append(xt)
                bt = const_pool.tile([P, 1], f32, tag=f"bias{h}")
                nc.sync.dma_start(bt[:], bias_dram[h * P : (h + 1) * P, :])
                bias_sb.append(bt)

            pi_half = const_pool.tile([P, 1], f32, tag="pi_half")
            nc.vector.memset(pi_half[:], PI / 2)

            # 4 PSUM accumulators: (j_half, b_half), each [128, 512] f32 = 1 bank
            accs = [
                [
                    psum_pool.tile(
                        [P, 512], f32, tag=f"acc{j}{b}", name=f"acc{j}{b}"
                    )
                    for b in range(2)
                ]
                for j in range(2)
            ]

            # Iteration order: chains (m, 2m, 4m) for odd m (depth<=2
            # angle doubling on DVE), then k with v2(k)>=3 direct.  Each
            # chain element after the first derives its trig tiles from the
            # immediately preceding (k, ih) iteration via s2k = 2*s*c,
            # c2k = 1 - 2*s^2 (bf16), skipping ACT and range reduction.
            order = []  # (k0 index, doubled: bool)
            for m in range(1, K + 1, 2):
                order.append((m - 1, False))
                if 2 * m <= K:
                    order.append((2 * m - 1, True))
                if 4 * m <= K:
                    order.append((4 * m - 1, True))
            for m in range(8, K + 1, 8):
                order.append((m - 1, False))
            assert sorted(k for k, _ in order) == list(range(K))

            prev_tiles = {}  # ih -> (s_tile, c_tile) of previous chain elem
            n_iter = K * 2
            it = -1
            for k, doubled in order:
              for ih in range(2):
                it += 1
                row0 = k * I + ih * P

                wc = w_pool.tile([P, J], bf16, tag="wc")
                nc.sync.dma_start(wc[:], w_dram[row0 : row0 + P, :])
                ws = w_pool.tile([P, J], bf16, tag="ws")
                nc.sync.dma_start(ws[:], w_dram[I * K + row0 : I * K + row0 + P, :])

                if doubled:
                    ps, pc = prev_tiles[ih]
                    sq = red_pool.tile([P, B], bf16, tag="sq")
                    nc.vector.tensor_tensor(sq[:], ps[:], ps[:], Alu.mult)
                    c_t = trig_pool.tile([P, B], bf16, tag="c_t")
                    nc.vector.tensor_scalar(c_t[:], sq[:], -2.0, 1.0, Alu.mult, Alu.add)
                    sc = red_pool.tile([P, B], bf16, tag="sc")
                    nc.vector.tensor_tensor(sc[:], ps[:], pc[:], Alu.mult)
                    s_t = trig_pool.tile([P, B], bf16, tag="s_t")
                    nc.vector.tensor_scalar(s_t[:], sc[:], 2.0, None, Alu.mult)
                else:
                    # range reduction via round-to-nearest magic trick:
                    # u = x*(k+1)/(2pi); v = round(u); f = u - v in [-.5, .5];
                    # sin(kx) = Sin(2pi*f).  |f| by clearing the sign bit;
                    # cos(kx) = cos(2pi*|f|) = Sin(pi/2 - 2pi*|f|).
                    u = red_pool.tile([P, B], f32, tag="u")
                    nc.vector.tensor_scalar(
                        u[:], xT_sb[ih][:], float((k + 1) / TWO_PI), None, Alu.mult
                    )
                    v = red_pool.tile([P, B], f32, tag="v")
                    nc.vector.tensor_scalar(
                        v[:], u[:], MAGIC, MAGIC, Alu.add, Alu.subtract
                    )
                    f = red_pool.tile([P, B], f32, tag="f")
                    nc.vector.tensor_tensor(f[:], u[:], v[:], Alu.subtract)
                    af = red_pool.tile([P, B], f32, tag="af")
                    nc.vector.tensor_scalar(
                        af[:].bitcast(mybir.dt.uint32),
                        f[:].bitcast(mybir.dt.uint32),
                        0x7FFFFFFF,
                        None,
                        Alu.bitwise_and,
                    )
                    s_t = trig_pool.tile([P, B], bf16, tag="s_t")
                    nc.scalar.activation(s_t[:], f[:], Act.Sin, bias=0.0, scale=TWO_PI)
                    c_t = trig_pool.tile([P, B], bf16, tag="c_t")
                    nc.scalar.activation(
                        c_t[:], af[:], Act.Sin, bias=pi_half[:], scale=-TWO_PI
                    )
                prev_tiles[ih] = (s_t, c_t)

                first = it == 0
                last = it == n_iter - 1
                for w_t, t_t, is_cos in ((wc, c_t, True), (ws, s_t, False)):
                    for j in range(2):
                        for b in range(2):
                            nc.tensor.matmul(
                                accs[j][b][:],
                                w_t[:, j * P : (j + 1) * P],
                                t_t[:, b * 512 : (b + 1) * 512],
                                start=(first and is_cos),
                                stop=(last and not is_cos),
                            )

            # Evacuate PSUM -> SBUF (add bias per partition) -> DRAM
            for j in range(2):
                o = out_pool.tile([P, B], f32, tag="o")
                for b in range(2):
                    nc.vector.tensor_scalar(
                        o[:, b * 512 : (b + 1) * 512],
                        accs[j][b][:],
                        bias_sb[j][:],
                        None,
                        Alu.add,
                    )
                nc.sync.dma_start(yT_dram[j * P : (j + 1) * P, :], o[:])

    nc.compile()
    _cache["nc"] = nc
    return nc


def _prep_w(fouriercoeffs: np.ndarray) -> np.ndarray:
    # fouriercoeffs: (2, J, I, K) f32 -> (2*K*I, J) bf16 with row order
    # (trig, k, i): row[t*K*I + k*I + i] = fouriercoeffs[t, :, i, k]
    w = np.ascontiguousarray(
        fouriercoeffs.transpose(0, 3, 2, 1).reshape(2 * K * I, J)
    )
    return w.astype(ml_dtypes.bfloat16)


def kernel(x: np.ndarray, fouriercoeffs: np.ndarray, bias: np.ndarray) -> np.ndarray:
    x = np.asarray(x, dtype=np.float32)
    fouriercoeffs = np.asarray(fouriercoeffs, dtype=np.float32)
    bias = np.asarray(bias, dtype=np.float32)

    nc = _build()
    w_host = _prep_w(fouriercoeffs)
    bias_col = np.ascontiguousarray(bias.reshape(J, 1))

    in_maps = []
    for c in range(N_CORES):
        shard = np.ascontiguousarray(x[c * B : (c + 1) * B].T)  # (I, B)
        in_maps.append({"xT": shard, "w": w_host, "bias": bias_col})

    res = bass_utils.run_bass_kernel_spmd(nc, in_maps, core_ids=list(range(N_CORES)))

    y = np.empty((B_FULL, J), dtype=np.float32)
    for c in range(N_CORES):
        y[c * B : (c + 1) * B] = res.results[c]["yT"].T
    return y


def profile_run(inputs):
    """Run once with NTFF tracing enabled; returns BassKernelResults."""
    x = np.asarray(inputs["x"], dtype=np.float32)
    nc = _build()
    w_host = _prep_w(np.asarray(inputs["fouriercoeffs"], dtype=np.float32))
    bias_col = np.ascontiguousarray(
        np.asarray(inputs["bias"], dtype=np.float32).reshape(J, 1)
    )
    in_maps = [
        {
            "xT": np.ascontiguousarray(x[c * B : (c + 1) * B].T),
            "w": w_host,
            "bias": bias_col,
        }
        for c in range(N_CORES)
    ]
    return bass_utils.run_bass_kernel_spmd(
        nc, in_maps, core_ids=list(range(N_CORES)), trace=True
    )

